# revision 1
# baseline (speedup 1.0000x reference)
"""Per-edge dot-product (GNN DotProductPredictor) Bass kernel for 8 trn2 cores.

score[e] = sum_k h[src[e], k] * h[dst[e], k]

v2 strategy — all random access happens in SBUF, not HBM (concurrent
random 256B HBM gathers collapse to ~22 GB/s chip-wide; SBUF-source
gathers run per-core and scale):

  - h is cast to fp16 and pair-packed: block b = [h[2b] | h[2b+1]]
    (256B). Block b lives at SBUF partition b%128, free range
    [256*(b//128), +256). Whole table = 12.8MB (~100KB/partition),
    loaded once per run by a single sequential DMA.
  - Edges are sharded contiguously across the 8 cores (400k each).
    Per core, edges are bucketed by (src bank, src parity, dst bank,
    dst parity): 2 banks because gather indices are int16 (<=32768
    blocks per view), parity selects which packed half is the row.
  - Per chunk of 4096 edges: idx-tile DMA, two SBUF-source dma_gathers
    (transpose mode) -> A,B [128, 4096] fp16 feature-major (partition p
    = fp16 word p of the 256B block; partition halves = the two packed
    nodes), DVE multiply (aligned-parity classes full-width; mixed
    classes re-base one half with an ACT copy first — TensorTensor
    requires equal SBUF partition bases), then a PE partition-reduce:
    [64,128] product slices as stationary x a ones vector -> psum
    [128, 32] = scores.
  - Device scores land at out[chunk*4096 + slot] (identity mapping);
    the host inverse-permutes to edge order. Bucket overflow beyond
    the static class capacity (never happens for uniform inputs at
    the chosen caps) falls back to numpy on the host.
"""

import contextlib
import re

import numpy as np

P = 128
D = 64

N_NODES = 100000
N_EDGES = 3200000
N_CORES = 8
EPC = N_EDGES // N_CORES

CHUNK = 4096
IDXC = CHUNK // 16
BANK0_BLOCKS = 32768

NBLK = (N_NODES + 1) // 2
RANKS = (NBLK + P - 1) // P
NBLK_PAD = RANKS * P
BANK0_RANKS = BANK0_BLOCKS // P

# class capacity in slots (multiple of 128). Uniform-random expectation
# per class per core: (b0,b0) 42949 (cap +5.5σ), mixed 22587 (+5.7σ),
# (b1,b1) 11878 (+6.2σ); overflow falls back to exact host numpy.
_CAPS = {(0, 0): 44032, (0, 1): 23424, (1, 0): 23424, (1, 1): 12544}

CLASSES = [
    (bs, ps, bd, pd, _CAPS[(bs, bd)])
    for bs in range(2)
    for ps in range(2)
    for bd in range(2)
    for pd in range(2)
]
# device chunk plan: per class, CHUNK-sized pieces + a variable tail
CHUNK_PLAN = []
for _bs, _ps, _bd, _pd, _cap in CLASSES:
    _left = _cap
    while _left > 0:
        _n = min(CHUNK, _left)
        CHUNK_PLAN.append((_bs, _ps, _bd, _pd, _n))
        _left -= _n
CAP = sum(c[-1] for c in CHUNK_PLAN)
IDX_ELEMS = sum(P * 2 * (c[-1] // 16) for c in CHUNK_PLAN)

_NC = None


def _build_nc(reps=1, bufs=2, queue_plan=None, record=None):
    import concourse.bacc as bacc
    import concourse.tile as tile
    from concourse import mybir

    nc = bacc.Bacc("TRN2", target_bir_lowering=False)
    tab_t = nc.dram_tensor(
        "table", [P, NBLK_PAD], mybir.dt.float16, kind="ExternalInput"
    )
    idx_t = nc.dram_tensor(
        "idx", [IDX_ELEMS], mybir.dt.int16, kind="ExternalInput"
    )
    out_t = nc.dram_tensor("out", [CAP], mybir.dt.float32, kind="ExternalOutput")
    table = nc.alloc_sbuf_tensor("tab_sb", [P, NBLK_PAD], mybir.dt.float16)

    b0e = BANK0_RANKS * P
    bank_views = [table[:, :b0e], table[:, b0e:]]

    with tile.TileContext(nc) as tc:
        nc.sync.dma_start(out=table[:, :], in_=tab_t[:, :])
        with tc.tile_pool(name="singles", bufs=1) as singles:
            ones = singles.tile([P, 1], mybir.dt.float16)
            nc.any.memset(ones[:], 1.0)
            loop = tc.For_i(0, reps, 1) if reps > 1 else contextlib.nullcontext()
            with (
                loop,
                tc.tile_pool(name="pool", bufs=bufs) as pool,
                tc.tile_pool(name="psum", bufs=bufs, space="PSUM") as psum_pool,
            ):
                iofs = 0
                oofs = 0
                for gi, (bs, ps, bd, pd, n) in enumerate(CHUNK_PLAN):
                    nix = n // 16
                    mcols = n // P
                    idxt = pool.tile([P, 2 * IDXC], mybir.dt.int16, tag="idx")
                    nc.sync.dma_start(
                        out=idxt[:, : 2 * nix],
                        in_=idx_t[iofs : iofs + P * 2 * nix].rearrange(
                            "(p c) -> p c", p=P
                        ),
                    )
                    A = pool.tile([P, 1, CHUNK], mybir.dt.float16, tag="A")
                    B = pool.tile([P, 1, CHUNK], mybir.dt.float16, tag="B")
                    qa = queue_plan[2 * gi] if queue_plan else 0
                    qb = queue_plan[2 * gi + 1] if queue_plan else 0
                    i1 = nc.gpsimd.dma_gather(
                        A[:, :, :n], bank_views[bs], idxt[:, :nix],
                        n, n, 2 * D,
                        transpose=True, single_packet=False,
                        sbuf_tokens_per_rank=P, sbuf_free_dim_per_rank=4 * D,
                        queue_num=qa,
                    )
                    i2 = nc.gpsimd.dma_gather(
                        B[:, :, :n], bank_views[bd], idxt[:, nix : 2 * nix],
                        n, n, 2 * D,
                        transpose=True, single_packet=False,
                        sbuf_tokens_per_rank=P, sbuf_free_dim_per_rank=4 * D,
                        queue_num=qb,
                    )
                    if record is not None:
                        record.extend([i1, i2])
                    pr = pool.tile([P, CHUNK], mybir.dt.float16, tag="pr")
                    lo = D * ps
                    if ps == pd:
                        nc.vector.tensor_tensor(
                            out=pr[:, :n], in0=A[:, 0, :n], in1=B[:, 0, :n],
                            op=mybir.AluOpType.mult,
                        )
                    else:
                        alg = pool.tile([P, CHUNK], mybir.dt.float16, tag="alg")
                        nc.scalar.copy(
                            out=alg[lo : lo + D, :n],
                            in_=B[D * pd : D * pd + D, 0, :n],
                        )
                        nc.vector.tensor_tensor(
                            out=pr[lo : lo + D, :n],
                            in0=A[lo : lo + D, 0, :n],
                            in1=alg[lo : lo + D, :n],
                            op=mybir.AluOpType.mult,
                        )
                    pst = psum_pool.tile([P, CHUNK // P], mybir.dt.float32)
                    for m in range(mcols):
                        nc.tensor.matmul(
                            pst[:, m : m + 1],
                            pr[lo : lo + D, m * P : (m + 1) * P],
                            ones[lo : lo + D, :],
                            start=True, stop=True,
                        )
                    sc = pool.tile([P, CHUNK // P], mybir.dt.float32, tag="sc")
                    nc.any.tensor_copy(sc[:, :mcols], pst[:, :mcols])
                    nc.sync.dma_start(
                        out=out_t[oofs : oofs + n].rearrange("(m p) -> p m", p=P),
                        in_=sc[:, :mcols],
                    )
                    iofs += P * 2 * nix
                    oofs += n
    nc.compile()
    return nc


def _sem_queue(inst):
    name = inst.ins.sync_info.on_update[0].ant_name
    return int(re.search(r"DMASW(\d+)", name).group(1)) % 4


def _build_best(reps=1, bufs=2):
    """Single-queue build. A two-pass multi-queue variant (queue_num =
    assigned DMASW sem % 4) was tried: ~2x faster gathers, sim-clean
    sem/queue partitioning, but it still produced nondeterministic
    wrong results on hardware — the SWDGE multi-queue path races
    beyond what the sem mapping controls. Do not re-enable."""
    return _build_nc(reps, bufs)


def _make_table(h_f16):
    blocks = np.zeros((NBLK_PAD, 2 * D), np.float16)
    flat = h_f16.reshape(-1)
    blocks.reshape(-1)[: flat.size] = flat
    return blocks.reshape(RANKS, P, 2 * D).transpose(1, 0, 2).reshape(P, NBLK_PAD)


def _wrap_one(loc):
    n = loc.shape[0]
    a = loc.reshape(n // 16, 16).T  # [16, n/16]
    return np.broadcast_to(a[None], (8, 16, n // 16)).reshape(P, n // 16)


def _prep_core(src_c, dst_c):
    blk_s, par_s = src_c >> 1, src_c & 1
    blk_d, par_d = dst_c >> 1, dst_c & 1
    bank_s = (blk_s >= BANK0_BLOCKS).astype(np.int64)
    bank_d = (blk_d >= BANK0_BLOCKS).astype(np.int64)
    loc_s = (blk_s - bank_s * BANK0_BLOCKS).astype(np.int16)
    loc_d = (blk_d - bank_d * BANK0_BLOCKS).astype(np.int16)
    cls = ((bank_s * 2 + par_s) * 4 + bank_d * 2 + par_d).astype(np.int64)
    order = np.argsort(cls, kind="stable")
    counts = np.bincount(cls, minlength=16)

    s_flat = np.zeros(CAP, np.int16)
    d_flat = np.zeros(CAP, np.int16)
    n = src_c.shape[0]
    edge_pos = np.full(n, -1, np.int64)
    overflow = np.zeros(n, bool)

    start = 0
    base = 0
    for bs, ps, bd, pd, cap in CLASSES:
        ci = (bs * 2 + ps) * 4 + bd * 2 + pd
        m = int(counts[ci])
        take = min(m, cap)
        e = order[start : start + take]
        slots = base + np.arange(take)
        s_flat[slots] = loc_s[e]
        d_flat[slots] = loc_d[e]
        edge_pos[e] = slots
        if m > take:
            overflow[order[start + take : start + m]] = True
        start += m
        base += cap

    # per chunk: [P, src cols | dst cols] row-major — must match the
    # device's idxt DMA rearrange("(p c) -> p c")
    parts = []
    ofs = 0
    for bs, ps, bd, pd, nsz in CHUNK_PLAN:
        ws = _wrap_one(s_flat[ofs : ofs + nsz])
        wd = _wrap_one(d_flat[ofs : ofs + nsz])
        parts.append(np.concatenate([ws, wd], axis=1).reshape(-1))
        ofs += nsz
    idx_dev = np.concatenate(parts)
    return idx_dev, edge_pos, overflow


def kernel(h, src, dst):
    global _NC
    from concourse import bass_utils

    h = np.ascontiguousarray(np.asarray(h), dtype=np.float32)
    src = np.asarray(src).astype(np.int64)
    dst = np.asarray(dst).astype(np.int64)

    if _NC is None:
        _NC = _build_best()

    table = _make_table(h.astype(np.float16))
    in_maps, maps = [], []
    for c in range(N_CORES):
        lo = c * EPC
        idx_dev, edge_pos, overflow = _prep_core(src[lo : lo + EPC], dst[lo : lo + EPC])
        in_maps.append({"table": table, "idx": idx_dev})
        maps.append((edge_pos, overflow))

    res = bass_utils.run_bass_kernel_spmd(
        _NC, in_maps, core_ids=list(range(N_CORES))
    )

    out = np.empty(N_EDGES, np.float32)
    for c in range(N_CORES):
        lo = c * EPC
        edge_pos, overflow = maps[c]
        dev_out = res.results[c]["out"]
        ok = ~overflow
        out[lo : lo + EPC][ok] = dev_out[edge_pos[ok]]
        if overflow.any():  # static capacity exceeded: exact host fallback
            e = np.nonzero(overflow)[0]
            s_ = src[lo : lo + EPC][e]
            d_ = dst[lo : lo + EPC][e]
            out[lo : lo + EPC][e] = np.einsum("ij,ij->i", h[s_], h[d_])
    return out.reshape(N_EDGES, 1)

